# revision 35
# baseline (speedup 1.0000x reference)
"""Trainium2 Bass kernel for nn_CoreAttention (S=2048, B=1, H=16, D=128).

Sharding: 16 heads across 8 NeuronCores (2 heads/core, tensor parallel).

Per head, fully fused causal attention:
    M      = Wqk Wqk^T                  (PE, one matmul; M is symmetric)
    kmt    = M K^T                      (PE, 4 matmuls ping-ponged through a
                                         1-bank PSUM scratch; q side stays RAW)
    v      = V_block @ Wv               (PE, 16 matmuls -> [s,e] chunks)
    scoresT[k,q] = kmt_j^T @ Q^T        (PE, causal only, streams into two
                                         big PSUM spans: P=4 banks, Q=2)
    expT   = exp(scoresT / NF)          (ACT, 12 big instrs/head, -> SBUF)
    mask   = affine_select on diagonal  (GPSIMD, zero strict lower triangle)
    ctx[q,(e|sum)] = sum_j expT_j^T @ [v_j | 1]   (PE, expT-stationary,
                                         129-wide rhs; col 128 = softmax sum)
    out    = ctx * (1/sum)              (DVE reciprocal + per-partition mul)

The two heads are software-pipelined: head1's projections are interleaved
into head0's span loop so ACT/PE never drain at the head seam. Input DMAs
are issued up-front on two queues (sync + gpsimd). No transposes, no
separate softmax-sum pass, no device-side casts (host supplies bf16
pre-transposed tensors). exp runs without max-subtraction: scores/NF ~
N(0,1), so exp stays in [e-6, e+6].
"""

import sys
from contextlib import ExitStack

import numpy as np

for _p in ("/opt/trn_rl_repo",):
    if _p not in sys.path:
        sys.path.insert(0, _p)

import ml_dtypes
import concourse.bass as bass
import concourse.tile as tile
from concourse import bacc, mybir
from concourse.bass_utils import run_bass_kernel_spmd

S, B, H, D = 2048, 1, 16, 128
HPC = 2  # heads per core
NCORES = 8
NB = S // 128  # 16 k-blocks of 128
NF = float(np.sqrt(2048.0 / 16.0))  # NORM_FACTOR
TOT = 17408  # total causal score columns

F32 = mybir.dt.float32
BF16 = mybir.dt.bfloat16
AF = mybir.ActivationFunctionType

# block start offsets in the concatenated causal score stream
OFF = [0]
for j in range(NB):
    OFF.append(OFF[-1] + (S - 128 * j))
assert OFF[-1] == TOT


def make_spans(split_last: bool):
    """(region_idx, region_off, start, len) alternating P(2048) / Q(1024)."""
    sizes = [2048, 1024] * 4 + [2048, 1024, 1024, 1024]
    spans = []
    pos = 0
    for i, ln in enumerate(sizes):
        r = i % 2
        if r == 0 and ln > 2048:
            ln = 2048
        spans.append([r, 0, pos, ln])
        pos += ln
    assert pos == TOT, pos
    if split_last:
        r, ro, st, ln = spans.pop()
        spans.append([r, 0, st, 512])
        spans.append([r, 512, st + 512, 512])
    return spans


def build_program() -> bass.Bass:
    nc = bacc.Bacc(
        "TRN2", target_bir_lowering=False, debug=False, num_devices=NCORES
    )

    qt_d = nc.dram_tensor("qt", [HPC, D, S], BF16, kind="ExternalInput")
    kt_d = nc.dram_tensor("kt", [HPC, D, S], BF16, kind="ExternalInput")
    vt_d = nc.dram_tensor("vt", [HPC, D, S], BF16, kind="ExternalInput")
    wqkt_d = nc.dram_tensor("wqkt", [HPC, D, D], BF16, kind="ExternalInput")
    wv_d = nc.dram_tensor("wv", [HPC, D, D], BF16, kind="ExternalInput")
    out_d = nc.dram_tensor("out", [HPC, S, D], F32, kind="ExternalOutput")

    with tile.TileContext(nc) as tc, ExitStack() as ctx:
        sb = ctx.enter_context(tc.tile_pool(name="sb", bufs=1))
        ps = ctx.enter_context(tc.tile_pool(name="ps", bufs=1, space="PSUM"))

        # warm the exp activation-table load under the initial DMAs
        warm = sb.tile([D, 1], F32, tag="warm")
        nc.gpsimd.memset(warm[:], 0.0)
        warm2 = sb.tile([D, 1], BF16, tag="warm2")
        nc.scalar.activation(warm2[:], warm[:], AF.Exp)
        # PE warmup tile: junk matmuls during the input-DMA window keep the
        # HAM clock gate at full rate so the first real matmuls run at 2.4GHz
        wup = sb.tile([D, D], BF16, tag="wup")
        nc.gpsimd.memset(wup[:], 0.0)

        # PSUM: P=4 banks, Q=2 banks, VP scratch=1 bank, ctx=1 bank
        P = ps.tile([D, 2048], F32, tag="P")
        Qr = ps.tile([D, 1024], F32, tag="Q")
        VP = ps.tile([D, 512], F32, tag="VP")
        ctxb = ps.tile([D, 3 * 129], F32, tag="ctx")
        regions = (P, Qr)

        class HeadEmitter:
            def __init__(self, h):
                self.h = h
                self.spans = make_spans(split_last=True)
                self.span_i = 0
                self.next_pair = [0] * NB
                self.closed = [False] * NB
                self.vrounds = 0
                self.bank_open = {}
                self.osb = None
                self.wqkt = sb.tile([D, D], BF16, tag="wqkt", bufs=2,
                                    name=f"wqkt_{h}")
                self.ktb = sb.tile([D, S], BF16, tag="ktb", bufs=2,
                                   name=f"ktb_{h}")
                self.qtb = sb.tile([D, S], BF16, tag="qtb", bufs=2,
                                   name=f"qtb_{h}")
                self.wvb = sb.tile([D, D], BF16, tag="wvb", bufs=2,
                                   name=f"wvb_{h}")
                self.vtb = sb.tile([D, S], BF16, tag="vtb", bufs=2,
                                   name=f"vtb_{h}")
                self.Mb = sb.tile([D, D], BF16, tag="Mb", bufs=2,
                                  name=f"Mb_{h}")
                self.kmt = sb.tile([D, S], BF16, tag="kmt", bufs=2,
                                   name=f"kmt_{h}")
                self.vsb = sb.tile([D, NB * 129], BF16, tag="vsb", bufs=2,
                                   name=f"vsb_{h}")
                self.vsb3 = self.vsb.rearrange("p (j e) -> p j e", j=NB)
                nc.gpsimd.memset(self.vsb3[:, :, 128:129], 1.0)
                self.expt = sb.tile([D, TOT], BF16, tag="expt", bufs=2,
                                    name=f"expt_{h}")

            def load(self):
                h = self.h
                # first k chunk gates the whole kproj->span0->exp0 chain
                nc.sync.dma_start(self.ktb[:, 0:1024], kt_d[h][:, 0:1024])
                nc.sync.dma_start(self.wqkt[:], wqkt_d[h])
                nc.sync.dma_start(self.ktb[:, 1024:2048], kt_d[h][:, 1024:2048])
                # head0's q load rides the otherwise-idle scalar queue so
                # span0 scores can start as early as possible
                qeng = nc.scalar if h == 0 else nc.sync
                for c in range(2):
                    sl = slice(1024 * c, 1024 * (c + 1))
                    qeng.dma_start(self.qtb[:, sl], qt_d[h][:, sl])

            def load_v(self):
                h = self.h
                # head0's v loads go LAST on the sync queue: the serialized
                # issue order gives the k/q transfers (which gate the first
                # exp) the DMA bandwidth to themselves
                veng = nc.sync if h == 0 else nc.gpsimd
                veng.dma_start(self.wvb[:], wv_d[h])
                for c in range(2):
                    sl = slice(1024 * c, 1024 * (c + 1))
                    veng.dma_start(self.vtb[:, sl], vt_d[h][:, sl])

            def pro_M(self):
                nc.tensor.matmul(VP[:, 0:128], self.wqkt[:], self.wqkt[:])
                nc.vector.tensor_copy(self.Mb[:], VP[:, 0:128])

            def kchunk(self, c):
                if c == 0:
                    # split so span0 (which only needs kmt[:, 0:128]) can
                    # start as soon as the first 128 columns are projected
                    nc.tensor.matmul(
                        VP[:, 384:512], self.Mb[:], self.ktb[:, 0:128]
                    )
                    nc.vector.tensor_copy(self.kmt[:, 0:128], VP[:, 384:512])
                    nc.tensor.matmul(
                        VP[:, 0:384], self.Mb[:], self.ktb[:, 128:512]
                    )
                    nc.vector.tensor_copy(self.kmt[:, 128:512], VP[:, 0:384])
                    return
                sl = slice(512 * c, 512 * (c + 1))
                nc.tensor.matmul(VP[:], self.Mb[:], self.ktb[:, sl])
                nc.vector.tensor_copy(self.kmt[:, sl], VP[:])

            def vround(self, r):
                for m in range(4):
                    j = 4 * r + m
                    nc.tensor.matmul(
                        VP[:, 128 * m : 128 * (m + 1)],
                        self.vtb[:, 128 * j : 128 * (j + 1)],
                        self.wvb[:],
                    )
                nc.vector.tensor_copy(
                    self.vsb3[:, 4 * r : 4 * r + 4, 0:128],
                    VP.rearrange("p (j e) -> p j e", j=4),
                )
                self.vrounds = r + 1

            def _fire_span(self, si):
                r, ro, st, ln = self.spans[si]
                reg = regions[r]
                nc.scalar.activation(
                    self.expt[:, st : st + ln],
                    reg[:, ro : ro + ln],
                    AF.Exp,
                    scale=1.0 / NF,
                )
                for j in range(NB):
                    if st <= OFF[j] < st + ln:
                        nc.gpsimd.affine_select(
                            self.expt[:, OFF[j] : OFF[j] + 128],
                            self.expt[:, OFF[j] : OFF[j] + 128],
                            pattern=[[1, 128]],
                            compare_op=mybir.AluOpType.is_ge,
                            fill=0.0,
                            base=0,
                            channel_multiplier=-1,
                        )

            def _slot(self, i):
                """PSUM slot for region i. Head1's later regions alternate
                between the ctx bank and the (by then idle) VP bank so two
                accumulation groups can be open concurrently."""
                if self.h == HPC - 1 and i >= 12:
                    bankid = 1 if i % 2 == 0 else 0
                    bank = VP if bankid else ctxb
                    col = ((i // 2) % 2) * 130
                    return (bankid, bank[:, col : col + 129])
                return (0, ctxb[:, (i % 3) * 129 : (i % 3) * 129 + 129])

            def _close_pv(self, i):
                h = self.h
                slot = self._slot(i)[1]
                if i % 4 == 0:
                    self.osb = sb.tile([D, 512], F32, tag="osb", bufs=2,
                                       name=f"osb_{h}_{i // 4}")
                    self.ctxs = sb.tile([D, 4 * 129], F32, tag="ctxs", bufs=2,
                                        name=f"ctxs_{h}_{i // 4}")
                # single fast copy releases the PSUM slot; normalization is
                # batched per group of 4 off the critical path
                ctxs3 = self.ctxs.rearrange("p (r e) -> p r e", r=4)
                nc.vector.tensor_copy(ctxs3[:, i % 4, :], slot)
                last_grp = (h == HPC - 1) and i >= 12
                if last_grp:
                    # final group of the final head: normalize + store per
                    # region so the kernel tail is as short as possible
                    r = i % 4
                    rec = sb.tile([D, 1], F32, tag="rec1", bufs=2,
                                  name=f"rec1_{h}_{i}")
                    nc.vector.reciprocal(rec[:], ctxs3[:, r, 128:129])
                    nc.vector.tensor_scalar_mul(
                        self.osb[:, r * 128 : r * 128 + 128],
                        ctxs3[:, r, 0:128],
                        rec[:],
                    )
                    # alternate issue queues so the tail DMAs pipeline
                    deng = (nc.sync, nc.scalar, nc.gpsimd, nc.sync)[r]
                    deng.dma_start(
                        out_d[h, 128 * i : 128 * (i + 1), :],
                        self.osb[:, r * 128 : r * 128 + 128],
                    )
                elif i % 4 == 3:
                    g = i // 4
                    rec = sb.tile([D, 4], F32, tag="rec", bufs=2,
                                  name=f"rec_{h}_{g}")
                    nc.vector.reciprocal(rec[:], ctxs3[:, :, 128])
                    for r in range(4):
                        nc.vector.tensor_scalar_mul(
                            self.osb[:, r * 128 : r * 128 + 128],
                            ctxs3[:, r, 0:128],
                            rec[:, r : r + 1],
                        )
                    nc.sync.dma_start(
                        out_d[h, 512 * g : 512 * (g + 1), :].rearrange(
                            "(b s) e -> s b e", b=4
                        ),
                        self.osb.rearrange("p (b e) -> p b e", b=4),
                    )

            def pv_flush(self):
                """Incrementally emit PV pairs whose expT weights and vsb
                chunks exist; a region's accumulation group stays open in its
                PSUM slot across spans. Region i may open once region i-3 has
                closed (3 rotating slots)."""
                done = self.spans[self.span_i - 1][2] + self.spans[self.span_i - 1][3] \
                    if self.span_i > 0 else 0
                for i in range(NB):
                    if self.closed[i]:
                        continue
                    # only ONE open accumulation group per 2KB PSUM bank:
                    # start=True marks the whole bank's zero-region on trn2,
                    # so a region may open only if its bank has no open group
                    bankid, slot = self._slot(i)
                    if bankid == 1 and self.vrounds < 4:
                        continue
                    cur = self.bank_open.get(bankid)
                    if cur is not None and cur != i:
                        continue
                    j = self.next_pair[i]
                    while (
                        j <= i
                        and OFF[j] + (i - j) * 128 + 128 <= done
                        and j < 4 * self.vrounds
                    ):
                        woff = OFF[j] + (i - j) * 128
                        nc.tensor.matmul(
                            slot,
                            self.expt[:, woff : woff + 128],
                            self.vsb[:, 129 * j : 129 * j + 129],
                            start=(j == 0),
                            stop=(j == i),
                        )
                        j += 1
                    self.next_pair[i] = j
                    if j > i:
                        self._close_pv(i)
                        self.closed[i] = True
                        self.bank_open[bankid] = None
                    elif j > 0:
                        self.bank_open[bankid] = i

            def span_run(self, a, b, flush=True):
                """Emit score matmuls + exp for spans [a, b)."""
                for si in range(a, b):
                    r, ro, st, ln = self.spans[si]
                    reg = regions[r]
                    pos = st
                    while pos < st + ln:
                        # current k-block
                        j = 0
                        while OFF[j + 1] <= pos:
                            j += 1
                        col = ro + (pos - st)
                        nxt = min(
                            OFF[j + 1],
                            st + ln,
                            pos + (512 - (col % 512)),
                        )
                        nc.tensor.matmul(
                            reg[:, col : col + (nxt - pos)],
                            self.kmt[:, 128 * j : 128 * (j + 1)],
                            self.qtb[
                                :,
                                128 * j + (pos - OFF[j]) : 128 * j + (nxt - OFF[j]),
                            ],
                        )
                        pos = nxt
                    self._fire_span(si)
                    self.span_i = si + 1
                    if flush:
                        self.pv_flush()

        e0 = HeadEmitter(0)
        e1 = HeadEmitter(1)

        # ---- software-pipelined drive of the two heads -------------------
        e0.load()
        # junk matmuls fill the PE during the input-DMA window (HAM warmup)
        for _ in range(30):
            nc.tensor.matmul(ctxb[:, 0:128], wup[:], wup[:])
        e0.pro_M()
        e0.kchunk(0)
        e0.load_v()
        e0.span_run(0, 1, flush=False)
        e0.vround(0)
        e0.kchunk(1)
        e0.span_run(1, 2)
        e0.vround(1)
        e0.span_run(2, 3)
        e0.vround(2)
        e0.span_run(3, 4)
        e0.vround(3)
        e0.span_run(4, 5)
        e0.kchunk(2)
        e0.span_run(5, 6)
        e0.kchunk(3)
        e1.load()
        e1.load_v()
        e0.span_run(6, 7)
        e1.pro_M()
        e0.span_run(7, 8)
        e1.kchunk(0)
        e0.span_run(8, 9)
        e1.kchunk(1)
        e0.span_run(9, 10)
        e1.kchunk(2)
        e0.span_run(10, 11)
        e1.kchunk(3)
        e0.span_run(11, 13)
        e1.span_run(0, 1, flush=False)
        e0.pv_flush()  # head0 tail regions
        e1.vround(0)
        e1.span_run(1, 2)
        e1.vround(1)
        e1.span_run(2, 3)
        e1.vround(2)
        e1.span_run(3, 4)
        e1.vround(3)
        e1.span_run(4, len(e1.spans))
        e1.pv_flush()

    nc.compile()
    return nc


_NC_CACHE = None


def _get_program():
    global _NC_CACHE
    if _NC_CACHE is None:
        _NC_CACHE = build_program()
    return _NC_CACHE


def make_in_maps(query_layer, key_layer, value_layer, svd_qk, svd_v):
    bf = ml_dtypes.bfloat16
    qt = np.ascontiguousarray(
        np.asarray(query_layer)[:, 0].transpose(1, 2, 0).astype(bf)
    )
    kt = np.ascontiguousarray(
        np.asarray(key_layer)[:, 0].transpose(1, 2, 0).astype(bf)
    )
    vt = np.ascontiguousarray(
        np.asarray(value_layer)[:, 0].transpose(1, 2, 0).astype(bf)
    )
    wqkt = np.ascontiguousarray(
        np.asarray(svd_qk).transpose(0, 2, 1).astype(bf)
    )
    wv = np.ascontiguousarray(np.asarray(svd_v).astype(bf))

    in_maps = []
    for c in range(NCORES):
        hs = slice(c * HPC, (c + 1) * HPC)
        in_maps.append(
            {
                "qt": qt[hs],
                "kt": kt[hs],
                "vt": vt[hs],
                "wqkt": wqkt[hs],
                "wv": wv[hs],
            }
        )
    return in_maps


def assemble_output(results):
    out = np.empty((S, B, H * D), dtype=np.float32)
    for c in range(NCORES):
        o = results[c]["out"]  # [HPC, S, D]
        for hl in range(HPC):
            h = c * HPC + hl
            out[:, 0, h * D : (h + 1) * D] = o[hl]
    return out


def kernel(query_layer, key_layer, value_layer, attention_mask, svd_qk, svd_v):
    nc = _get_program()
    in_maps = make_in_maps(query_layer, key_layer, value_layer, svd_qk, svd_v)
    res = run_bass_kernel_spmd(nc, in_maps, list(range(NCORES))).results
    return assemble_output(res)


# revision 37
# speedup vs baseline: 1.0373x; 1.0373x over previous
"""Trainium2 Bass kernel for nn_CoreAttention (S=2048, B=1, H=16, D=128).

Sharding: 16 heads across 8 NeuronCores (2 heads/core, tensor parallel).

Per head, fully fused causal attention:
    M      = Wqk Wqk^T                  (PE, one matmul; M is symmetric)
    kmt    = M K^T                      (PE, 4 matmuls ping-ponged through a
                                         1-bank PSUM scratch; q side stays RAW)
    v      = V_block @ Wv               (PE, 16 matmuls -> [s,e] chunks)
    scoresT[k,q] = kmt_j^T @ Q^T        (PE, causal only, streams into two
                                         big PSUM spans: P=4 banks, Q=2)
    expT   = exp(scoresT / NF)          (ACT, 12 big instrs/head, -> SBUF)
    mask   = affine_select on diagonal  (GPSIMD, zero strict lower triangle)
    ctx[q,(e|sum)] = sum_j expT_j^T @ [v_j | 1]   (PE, expT-stationary,
                                         129-wide rhs; col 128 = softmax sum)
    out    = ctx * (1/sum)              (DVE reciprocal + per-partition mul)

The two heads are software-pipelined: head1's projections are interleaved
into head0's span loop so ACT/PE never drain at the head seam. Input DMAs
are issued up-front on two queues (sync + gpsimd). No transposes, no
separate softmax-sum pass, no device-side casts (host supplies bf16
pre-transposed tensors). exp runs without max-subtraction: scores/NF ~
N(0,1), so exp stays in [e-6, e+6].
"""

import sys
from contextlib import ExitStack

import numpy as np

for _p in ("/opt/trn_rl_repo",):
    if _p not in sys.path:
        sys.path.insert(0, _p)

import ml_dtypes
import concourse.bass as bass
import concourse.tile as tile
from concourse import bacc, mybir
from concourse.bass_utils import run_bass_kernel_spmd

S, B, H, D = 2048, 1, 16, 128
HPC = 2  # heads per core
NCORES = 8
NB = S // 128  # 16 k-blocks of 128
NF = float(np.sqrt(2048.0 / 16.0))  # NORM_FACTOR
TOT = 17408  # total causal score columns

F32 = mybir.dt.float32
BF16 = mybir.dt.bfloat16
AF = mybir.ActivationFunctionType

# block start offsets in the concatenated causal score stream
OFF = [0]
for j in range(NB):
    OFF.append(OFF[-1] + (S - 128 * j))
assert OFF[-1] == TOT


def make_spans(split_last: bool):
    """(region_idx, region_off, start, len) alternating P / Q (1536 each)."""
    sizes = [1536] * 11 + [512]
    spans = []
    pos = 0
    for i, ln in enumerate(sizes):
        spans.append([i % 2, 0, pos, ln])
        pos += ln
    assert pos == TOT, pos
    return spans


def build_program() -> bass.Bass:
    nc = bacc.Bacc(
        "TRN2", target_bir_lowering=False, debug=False, num_devices=NCORES
    )

    qt_d = nc.dram_tensor("qt", [HPC, D, S], BF16, kind="ExternalInput")
    kt_d = nc.dram_tensor("kt", [HPC, D, S], BF16, kind="ExternalInput")
    vt_d = nc.dram_tensor("vt", [HPC, D, S], BF16, kind="ExternalInput")
    wqkt_d = nc.dram_tensor("wqkt", [HPC, D, D], BF16, kind="ExternalInput")
    wv_d = nc.dram_tensor("wv", [HPC, D, D], BF16, kind="ExternalInput")
    out_d = nc.dram_tensor("out", [HPC, S, D], F32, kind="ExternalOutput")

    with tile.TileContext(nc) as tc, ExitStack() as ctx:
        sb = ctx.enter_context(tc.tile_pool(name="sb", bufs=1))
        ps = ctx.enter_context(tc.tile_pool(name="ps", bufs=1, space="PSUM"))

        # warm the exp activation-table load under the initial DMAs
        warm = sb.tile([D, 1], F32, tag="warm")
        nc.gpsimd.memset(warm[:], 0.0)
        warm2 = sb.tile([D, 1], BF16, tag="warm2")
        nc.scalar.activation(warm2[:], warm[:], AF.Exp)
        # PE warmup tile: junk matmuls during the input-DMA window keep the
        # HAM clock gate at full rate so the first real matmuls run at 2.4GHz
        wup = sb.tile([D, D], BF16, tag="wup")
        nc.gpsimd.memset(wup[:], 0.0)

        # PSUM: P=3 banks, Q=3 banks, VP scratch=1 bank, ctx=1 bank
        P = ps.tile([D, 1536], F32, tag="P")
        Qr = ps.tile([D, 1536], F32, tag="Q")
        VP = ps.tile([D, 512], F32, tag="VP")
        ctxb = ps.tile([D, 3 * 129], F32, tag="ctx")
        regions = (P, Qr)

        class HeadEmitter:
            def __init__(self, h):
                self.h = h
                self.spans = make_spans(split_last=True)
                self.span_i = 0
                self.next_pair = [0] * NB
                self.closed = [False] * NB
                self.vrounds = 0
                self.bank_open = {}
                self.osb = None
                self.wqkt = sb.tile([D, D], BF16, tag="wqkt", bufs=2,
                                    name=f"wqkt_{h}")
                self.ktb = sb.tile([D, S], BF16, tag="ktb", bufs=2,
                                   name=f"ktb_{h}")
                self.qtb = sb.tile([D, S], BF16, tag="qtb", bufs=2,
                                   name=f"qtb_{h}")
                self.wvb = sb.tile([D, D], BF16, tag="wvb", bufs=2,
                                   name=f"wvb_{h}")
                self.vtb = sb.tile([D, S], BF16, tag="vtb", bufs=2,
                                   name=f"vtb_{h}")
                self.Mb = sb.tile([D, D], BF16, tag="Mb", bufs=2,
                                  name=f"Mb_{h}")
                self.kmt = sb.tile([D, S], BF16, tag="kmt", bufs=2,
                                   name=f"kmt_{h}")
                self.vsb = sb.tile([D, NB * 129], BF16, tag="vsb", bufs=2,
                                   name=f"vsb_{h}")
                self.vsb3 = self.vsb.rearrange("p (j e) -> p j e", j=NB)
                nc.gpsimd.memset(self.vsb3[:, :, 128:129], 1.0)
                self.expt = sb.tile([D, TOT], BF16, tag="expt", bufs=2,
                                    name=f"expt_{h}")

            def load(self):
                h = self.h
                # first k chunk gates the whole kproj->span0->exp0 chain
                nc.sync.dma_start(self.ktb[:, 0:1024], kt_d[h][:, 0:1024])
                nc.sync.dma_start(self.wqkt[:], wqkt_d[h])
                nc.sync.dma_start(self.ktb[:, 1024:2048], kt_d[h][:, 1024:2048])
                # head0's q load rides the otherwise-idle scalar queue so
                # span0 scores can start as early as possible
                qeng = nc.scalar if h == 0 else nc.sync
                for c in range(2):
                    sl = slice(1024 * c, 1024 * (c + 1))
                    qeng.dma_start(self.qtb[:, sl], qt_d[h][:, sl])

            def load_v(self):
                h = self.h
                # head0's v loads go LAST on the sync queue: the serialized
                # issue order gives the k/q transfers (which gate the first
                # exp) the DMA bandwidth to themselves
                veng = nc.sync if h == 0 else nc.gpsimd
                veng.dma_start(self.wvb[:], wv_d[h])
                for c in range(2):
                    sl = slice(1024 * c, 1024 * (c + 1))
                    veng.dma_start(self.vtb[:, sl], vt_d[h][:, sl])

            def pro_M(self):
                nc.tensor.matmul(VP[:, 0:128], self.wqkt[:], self.wqkt[:])
                nc.vector.tensor_copy(self.Mb[:], VP[:, 0:128])

            def kchunk(self, c):
                if c == 0:
                    # split so span0 (which only needs kmt[:, 0:128]) can
                    # start as soon as the first 128 columns are projected
                    nc.tensor.matmul(
                        VP[:, 384:512], self.Mb[:], self.ktb[:, 0:128]
                    )
                    nc.vector.tensor_copy(self.kmt[:, 0:128], VP[:, 384:512])
                    nc.tensor.matmul(
                        VP[:, 0:384], self.Mb[:], self.ktb[:, 128:512]
                    )
                    nc.vector.tensor_copy(self.kmt[:, 128:512], VP[:, 0:384])
                    return
                sl = slice(512 * c, 512 * (c + 1))
                nc.tensor.matmul(VP[:], self.Mb[:], self.ktb[:, sl])
                nc.vector.tensor_copy(self.kmt[:, sl], VP[:])

            def vround(self, r):
                for m in range(4):
                    j = 4 * r + m
                    nc.tensor.matmul(
                        VP[:, 128 * m : 128 * (m + 1)],
                        self.vtb[:, 128 * j : 128 * (j + 1)],
                        self.wvb[:],
                    )
                nc.vector.tensor_copy(
                    self.vsb3[:, 4 * r : 4 * r + 4, 0:128],
                    VP.rearrange("p (j e) -> p j e", j=4),
                )
                self.vrounds = r + 1

            def _fire_span(self, si):
                r, ro, st, ln = self.spans[si]
                reg = regions[r]
                nc.scalar.activation(
                    self.expt[:, st : st + ln],
                    reg[:, ro : ro + ln],
                    AF.Exp,
                    scale=1.0 / NF,
                )
                for j in range(NB):
                    if st <= OFF[j] < st + ln:
                        nc.gpsimd.affine_select(
                            self.expt[:, OFF[j] : OFF[j] + 128],
                            self.expt[:, OFF[j] : OFF[j] + 128],
                            pattern=[[1, 128]],
                            compare_op=mybir.AluOpType.is_ge,
                            fill=0.0,
                            base=0,
                            channel_multiplier=-1,
                        )

            def _slot(self, i):
                """PSUM slot for region i. Head1's later regions alternate
                between the ctx bank and the (by then idle) VP bank so two
                accumulation groups can be open concurrently."""
                if self.h == HPC - 1 and i >= 12:
                    bankid = 1 if i % 2 == 0 else 0
                    bank = VP if bankid else ctxb
                    col = ((i // 2) % 2) * 130
                    return (bankid, bank[:, col : col + 129])
                return (0, ctxb[:, (i % 3) * 129 : (i % 3) * 129 + 129])

            def _close_pv(self, i):
                h = self.h
                slot = self._slot(i)[1]
                if i % 4 == 0:
                    self.osb = sb.tile([D, 512], F32, tag="osb", bufs=2,
                                       name=f"osb_{h}_{i // 4}")
                    self.ctxs = sb.tile([D, 4 * 129], F32, tag="ctxs", bufs=2,
                                        name=f"ctxs_{h}_{i // 4}")
                # single fast copy releases the PSUM slot; normalization is
                # batched per group of 4 off the critical path
                ctxs3 = self.ctxs.rearrange("p (r e) -> p r e", r=4)
                nc.vector.tensor_copy(ctxs3[:, i % 4, :], slot)
                last_grp = (h == HPC - 1) and i >= 12
                if last_grp:
                    # final group of the final head: normalize + store per
                    # region so the kernel tail is as short as possible
                    r = i % 4
                    rec = sb.tile([D, 1], F32, tag="rec1", bufs=2,
                                  name=f"rec1_{h}_{i}")
                    nc.vector.reciprocal(rec[:], ctxs3[:, r, 128:129])
                    nc.vector.tensor_scalar_mul(
                        self.osb[:, r * 128 : r * 128 + 128],
                        ctxs3[:, r, 0:128],
                        rec[:],
                    )
                    # alternate issue queues so the tail DMAs pipeline
                    deng = (nc.sync, nc.scalar, nc.gpsimd, nc.sync)[r]
                    deng.dma_start(
                        out_d[h, 128 * i : 128 * (i + 1), :],
                        self.osb[:, r * 128 : r * 128 + 128],
                    )
                elif i % 4 == 3:
                    g = i // 4
                    rec = sb.tile([D, 4], F32, tag="rec", bufs=2,
                                  name=f"rec_{h}_{g}")
                    nc.vector.reciprocal(rec[:], ctxs3[:, :, 128])
                    for r in range(4):
                        nc.vector.tensor_scalar_mul(
                            self.osb[:, r * 128 : r * 128 + 128],
                            ctxs3[:, r, 0:128],
                            rec[:, r : r + 1],
                        )
                    nc.sync.dma_start(
                        out_d[h, 512 * g : 512 * (g + 1), :].rearrange(
                            "(b s) e -> s b e", b=4
                        ),
                        self.osb.rearrange("p (b e) -> p b e", b=4),
                    )

            def pv_flush(self):
                """Incrementally emit PV pairs whose expT weights and vsb
                chunks exist; a region's accumulation group stays open in its
                PSUM slot across spans. Region i may open once region i-3 has
                closed (3 rotating slots)."""
                done = self.spans[self.span_i - 1][2] + self.spans[self.span_i - 1][3] \
                    if self.span_i > 0 else 0
                for i in range(NB):
                    if self.closed[i]:
                        continue
                    # only ONE open accumulation group per 2KB PSUM bank:
                    # start=True marks the whole bank's zero-region on trn2,
                    # so a region may open only if its bank has no open group
                    bankid, slot = self._slot(i)
                    if bankid == 1 and self.vrounds < 4:
                        continue
                    cur = self.bank_open.get(bankid)
                    if cur is not None and cur != i:
                        continue
                    j = self.next_pair[i]
                    while (
                        j <= i
                        and OFF[j] + (i - j) * 128 + 128 <= done
                        and j < 4 * self.vrounds
                    ):
                        woff = OFF[j] + (i - j) * 128
                        nc.tensor.matmul(
                            slot,
                            self.expt[:, woff : woff + 128],
                            self.vsb[:, 129 * j : 129 * j + 129],
                            start=(j == 0),
                            stop=(j == i),
                        )
                        j += 1
                    self.next_pair[i] = j
                    if j > i:
                        self._close_pv(i)
                        self.closed[i] = True
                        self.bank_open[bankid] = None
                    elif j > 0:
                        self.bank_open[bankid] = i

            def span_run(self, a, b, flush=True):
                """Emit score matmuls + exp for spans [a, b)."""
                for si in range(a, b):
                    r, ro, st, ln = self.spans[si]
                    reg = regions[r]
                    pos = st
                    while pos < st + ln:
                        # current k-block
                        j = 0
                        while OFF[j + 1] <= pos:
                            j += 1
                        col = ro + (pos - st)
                        nxt = min(
                            OFF[j + 1],
                            st + ln,
                            pos + (512 - (col % 512)),
                        )
                        nc.tensor.matmul(
                            reg[:, col : col + (nxt - pos)],
                            self.kmt[:, 128 * j : 128 * (j + 1)],
                            self.qtb[
                                :,
                                128 * j + (pos - OFF[j]) : 128 * j + (nxt - OFF[j]),
                            ],
                        )
                        pos = nxt
                    self._fire_span(si)
                    self.span_i = si + 1
                    if flush:
                        self.pv_flush()

        e0 = HeadEmitter(0)
        e1 = HeadEmitter(1)

        # ---- software-pipelined drive of the two heads -------------------
        e0.load()
        # junk matmuls fill the PE during the input-DMA window (HAM warmup)
        for _ in range(30):
            nc.tensor.matmul(ctxb[:, 0:128], wup[:], wup[:])
        e0.pro_M()
        e0.kchunk(0)
        e0.load_v()
        e0.span_run(0, 1, flush=False)
        e0.vround(0)
        e0.kchunk(1)
        e0.span_run(1, 2)
        e0.vround(1)
        e0.span_run(2, 3)
        e0.vround(2)
        e0.span_run(3, 4)
        e0.vround(3)
        e0.span_run(4, 5)
        e0.kchunk(2)
        e0.span_run(5, 6)
        e0.kchunk(3)
        e1.load()
        e1.load_v()
        e0.span_run(6, 7)
        e1.pro_M()
        e0.span_run(7, 8)
        e1.kchunk(0)
        e0.span_run(8, 9)
        e1.kchunk(1)
        e0.span_run(9, 10)
        e1.kchunk(2)
        e0.span_run(10, 11)
        e1.kchunk(3)
        e0.span_run(11, 12)
        e1.span_run(0, 1, flush=False)
        e0.pv_flush()  # head0 tail regions
        e1.vround(0)
        e1.span_run(1, 2)
        e1.vround(1)
        e1.span_run(2, 3)
        e1.vround(2)
        e1.span_run(3, 4)
        e1.vround(3)
        e1.span_run(4, len(e1.spans))
        e1.pv_flush()

    nc.compile()
    return nc


_NC_CACHE = None


def _get_program():
    global _NC_CACHE
    if _NC_CACHE is None:
        _NC_CACHE = build_program()
    return _NC_CACHE


def make_in_maps(query_layer, key_layer, value_layer, svd_qk, svd_v):
    bf = ml_dtypes.bfloat16
    qt = np.ascontiguousarray(
        np.asarray(query_layer)[:, 0].transpose(1, 2, 0).astype(bf)
    )
    kt = np.ascontiguousarray(
        np.asarray(key_layer)[:, 0].transpose(1, 2, 0).astype(bf)
    )
    vt = np.ascontiguousarray(
        np.asarray(value_layer)[:, 0].transpose(1, 2, 0).astype(bf)
    )
    wqkt = np.ascontiguousarray(
        np.asarray(svd_qk).transpose(0, 2, 1).astype(bf)
    )
    wv = np.ascontiguousarray(np.asarray(svd_v).astype(bf))

    in_maps = []
    for c in range(NCORES):
        hs = slice(c * HPC, (c + 1) * HPC)
        in_maps.append(
            {
                "qt": qt[hs],
                "kt": kt[hs],
                "vt": vt[hs],
                "wqkt": wqkt[hs],
                "wv": wv[hs],
            }
        )
    return in_maps


def assemble_output(results):
    out = np.empty((S, B, H * D), dtype=np.float32)
    for c in range(NCORES):
        o = results[c]["out"]  # [HPC, S, D]
        for hl in range(HPC):
            h = c * HPC + hl
            out[:, 0, h * D : (h + 1) * D] = o[hl]
    return out


def kernel(query_layer, key_layer, value_layer, attention_mask, svd_qk, svd_v):
    nc = _get_program()
    in_maps = make_in_maps(query_layer, key_layer, value_layer, svd_qk, svd_v)
    res = run_bass_kernel_spmd(nc, in_maps, list(range(NCORES))).results
    return assemble_output(res)


# revision 38
# speedup vs baseline: 1.0721x; 1.0336x over previous
"""Trainium2 Bass kernel for nn_CoreAttention (S=2048, B=1, H=16, D=128).

Sharding: 16 heads across 8 NeuronCores (2 heads/core, tensor parallel).

Per head, fully fused causal attention:
    M      = Wqk Wqk^T                  (PE, one matmul; M is symmetric)
    kmt    = M K^T                      (PE, 4 matmuls ping-ponged through a
                                         1-bank PSUM scratch; q side stays RAW)
    v      = V_block @ Wv               (PE, 16 matmuls -> [s,e] chunks)
    scoresT[k,q] = kmt_j^T @ Q^T        (PE, causal only, streams into two
                                         big PSUM spans: P=4 banks, Q=2)
    expT   = exp(scoresT / NF)          (ACT, 12 big instrs/head, -> SBUF)
    mask   = affine_select on diagonal  (GPSIMD, zero strict lower triangle)
    ctx[q,(e|sum)] = sum_j expT_j^T @ [v_j | 1]   (PE, expT-stationary,
                                         129-wide rhs; col 128 = softmax sum)
    out    = ctx * (1/sum)              (DVE reciprocal + per-partition mul)

The two heads are software-pipelined: head1's projections are interleaved
into head0's span loop so ACT/PE never drain at the head seam. Input DMAs
are issued up-front on two queues (sync + gpsimd). No transposes, no
separate softmax-sum pass, no device-side casts (host supplies bf16
pre-transposed tensors). exp runs without max-subtraction: scores/NF ~
N(0,1), so exp stays in [e-6, e+6].
"""

import sys
from contextlib import ExitStack

import numpy as np

for _p in ("/opt/trn_rl_repo",):
    if _p not in sys.path:
        sys.path.insert(0, _p)

import ml_dtypes
import concourse.bass as bass
import concourse.tile as tile
from concourse import bacc, mybir
from concourse.bass_utils import run_bass_kernel_spmd

S, B, H, D = 2048, 1, 16, 128
HPC = 2  # heads per core
NCORES = 8
NB = S // 128  # 16 k-blocks of 128
NF = float(np.sqrt(2048.0 / 16.0))  # NORM_FACTOR
TOT = 17408  # total causal score columns

F32 = mybir.dt.float32
BF16 = mybir.dt.bfloat16
AF = mybir.ActivationFunctionType

# block start offsets in the concatenated causal score stream
OFF = [0]
for j in range(NB):
    OFF.append(OFF[-1] + (S - 128 * j))
assert OFF[-1] == TOT


def make_spans(split_last: bool):
    """(region_idx, region_off, start, len) alternating P / Q (1536 each)."""
    sizes = [1536] * 11 + [512]
    spans = []
    pos = 0
    for i, ln in enumerate(sizes):
        spans.append([i % 2, 0, pos, ln])
        pos += ln
    assert pos == TOT, pos
    return spans


def build_program() -> bass.Bass:
    nc = bacc.Bacc(
        "TRN2", target_bir_lowering=False, debug=False, num_devices=NCORES
    )

    qt_d = nc.dram_tensor("qt", [HPC, D, S], BF16, kind="ExternalInput")
    kt_d = nc.dram_tensor("kt", [HPC, D, S], BF16, kind="ExternalInput")
    vt_d = nc.dram_tensor("vt", [HPC, D, S], BF16, kind="ExternalInput")
    wqkt_d = nc.dram_tensor("wqkt", [HPC, D, D], BF16, kind="ExternalInput")
    wv_d = nc.dram_tensor("wv", [HPC, D, D], BF16, kind="ExternalInput")
    out_d = nc.dram_tensor("out", [HPC, S, D], F32, kind="ExternalOutput")

    with tile.TileContext(nc) as tc, ExitStack() as ctx:
        sb = ctx.enter_context(tc.tile_pool(name="sb", bufs=1))
        ps = ctx.enter_context(tc.tile_pool(name="ps", bufs=1, space="PSUM"))

        # warm the exp activation-table load under the initial DMAs
        warm = sb.tile([D, 1], F32, tag="warm")
        nc.gpsimd.memset(warm[:], 0.0)
        warm2 = sb.tile([D, 1], BF16, tag="warm2")
        nc.scalar.activation(warm2[:], warm[:], AF.Exp)
        # PE warmup tile: junk matmuls during the input-DMA window keep the
        # HAM clock gate at full rate so the first real matmuls run at 2.4GHz
        wup = sb.tile([D, D], BF16, tag="wup")
        nc.gpsimd.memset(wup[:], 0.0)

        # PSUM: P=3 banks, Q=3 banks, VP scratch=1 bank, ctx=1 bank
        P = ps.tile([D, 1536], F32, tag="P")
        Qr = ps.tile([D, 1536], F32, tag="Q")
        VP = ps.tile([D, 512], F32, tag="VP")
        ctxb = ps.tile([D, 3 * 129], F32, tag="ctx")
        regions = (P, Qr)

        class HeadEmitter:
            def __init__(self, h):
                self.h = h
                self.spans = make_spans(split_last=True)
                self.span_i = 0
                self.next_pair = [0] * NB
                self.closed = [False] * NB
                self.vrounds = 0
                self.bank_open = {}
                self.osb = None
                self.wqkt = sb.tile([D, D], BF16, tag="wqkt", bufs=2,
                                    name=f"wqkt_{h}")
                self.ktb = sb.tile([D, S], BF16, tag="ktb", bufs=2,
                                   name=f"ktb_{h}")
                self.qtb = sb.tile([D, S], BF16, tag="qtb", bufs=2,
                                   name=f"qtb_{h}")
                self.wvb = sb.tile([D, D], BF16, tag="wvb", bufs=2,
                                   name=f"wvb_{h}")
                self.vtb = sb.tile([D, S], BF16, tag="vtb", bufs=2,
                                   name=f"vtb_{h}")
                self.Mb = sb.tile([D, D], BF16, tag="Mb", bufs=2,
                                  name=f"Mb_{h}")
                self.kmt = sb.tile([D, S], BF16, tag="kmt", bufs=2,
                                   name=f"kmt_{h}")
                self.vsb = sb.tile([D, NB * 129], BF16, tag="vsb", bufs=2,
                                   name=f"vsb_{h}")
                self.vsb3 = self.vsb.rearrange("p (j e) -> p j e", j=NB)
                nc.gpsimd.memset(self.vsb3[:, :, 128:129], 1.0)
                self.expt = sb.tile([D, TOT], BF16, tag="expt", bufs=2,
                                    name=f"expt_{h}")

            def load(self):
                h = self.h
                # first k chunk gates the whole kproj->span0->exp0 chain
                nc.sync.dma_start(self.ktb[:, 0:1024], kt_d[h][:, 0:1024])
                nc.sync.dma_start(self.wqkt[:], wqkt_d[h])
                nc.sync.dma_start(self.ktb[:, 1024:2048], kt_d[h][:, 1024:2048])
                # head0's q load rides the otherwise-idle scalar queue so
                # span0 scores can start as early as possible
                qeng = nc.scalar if h == 0 else nc.sync
                for c in range(2):
                    sl = slice(1024 * c, 1024 * (c + 1))
                    qeng.dma_start(self.qtb[:, sl], qt_d[h][:, sl])

            def load_v(self):
                h = self.h
                # head0's v loads go LAST on the sync queue: the serialized
                # issue order gives the k/q transfers (which gate the first
                # exp) the DMA bandwidth to themselves
                veng = nc.sync if h == 0 else nc.gpsimd
                veng.dma_start(self.wvb[:], wv_d[h])
                for c in range(2):
                    sl = slice(1024 * c, 1024 * (c + 1))
                    veng.dma_start(self.vtb[:, sl], vt_d[h][:, sl])

            def pro_M(self):
                nc.tensor.matmul(VP[:, 0:128], self.wqkt[:], self.wqkt[:])
                nc.vector.tensor_copy(self.Mb[:], VP[:, 0:128])

            def kchunk(self, c):
                if c == 0:
                    # split so span0 (which only needs kmt[:, 0:128]) can
                    # start as soon as the first 128 columns are projected
                    nc.tensor.matmul(
                        VP[:, 384:512], self.Mb[:], self.ktb[:, 0:128]
                    )
                    nc.vector.tensor_copy(self.kmt[:, 0:128], VP[:, 384:512])
                    nc.tensor.matmul(
                        VP[:, 0:384], self.Mb[:], self.ktb[:, 128:512]
                    )
                    nc.vector.tensor_copy(self.kmt[:, 128:512], VP[:, 0:384])
                    return
                sl = slice(512 * c, 512 * (c + 1))
                nc.tensor.matmul(VP[:], self.Mb[:], self.ktb[:, sl])
                nc.vector.tensor_copy(self.kmt[:, sl], VP[:])

            def vround(self, r):
                for m in range(4):
                    j = 4 * r + m
                    nc.tensor.matmul(
                        VP[:, 128 * m : 128 * (m + 1)],
                        self.vtb[:, 128 * j : 128 * (j + 1)],
                        self.wvb[:],
                    )
                nc.vector.tensor_copy(
                    self.vsb3[:, 4 * r : 4 * r + 4, 0:128],
                    VP.rearrange("p (j e) -> p j e", j=4),
                )
                self.vrounds = r + 1

            def _fire_span(self, si):
                r, ro, st, ln = self.spans[si]
                reg = regions[r]
                nc.scalar.activation(
                    self.expt[:, st : st + ln],
                    reg[:, ro : ro + ln],
                    AF.Exp,
                    scale=1.0 / NF,
                )
                for j in range(NB):
                    if st <= OFF[j] < st + ln:
                        nc.gpsimd.affine_select(
                            self.expt[:, OFF[j] : OFF[j] + 128],
                            self.expt[:, OFF[j] : OFF[j] + 128],
                            pattern=[[1, 128]],
                            compare_op=mybir.AluOpType.is_ge,
                            fill=0.0,
                            base=0,
                            channel_multiplier=-1,
                        )

            def _slot(self, i):
                """PSUM slot for region i. Head1's later regions alternate
                between the ctx bank and the (by then idle) VP bank so two
                accumulation groups can be open concurrently."""
                if self.h == HPC - 1 and i >= 12:
                    bankid = 1 if i % 2 == 0 else 0
                    bank = VP if bankid else ctxb
                    col = ((i // 2) % 2) * 130
                    return (bankid, bank[:, col : col + 129])
                return (0, ctxb[:, (i % 3) * 129 : (i % 3) * 129 + 129])

            def _close_pv(self, i):
                h = self.h
                slot = self._slot(i)[1]
                if i % 4 == 0:
                    self.osb = sb.tile([D, 512], F32, tag="osb", bufs=2,
                                       name=f"osb_{h}_{i // 4}")
                    self.ctxs = sb.tile([D, 4 * 129], F32, tag="ctxs", bufs=2,
                                        name=f"ctxs_{h}_{i // 4}")
                # single fast copy releases the PSUM slot; normalization is
                # batched per group of 4 off the critical path
                ctxs3 = self.ctxs.rearrange("p (r e) -> p r e", r=4)
                nc.vector.tensor_copy(ctxs3[:, i % 4, :], slot)
                last_grp = (h == HPC - 1) and i >= 12
                if last_grp:
                    # final group of the final head: normalize + store per
                    # region so the kernel tail is as short as possible
                    r = i % 4
                    rec = sb.tile([D, 1], F32, tag="rec1", bufs=2,
                                  name=f"rec1_{h}_{i}")
                    nc.vector.reciprocal(rec[:], ctxs3[:, r, 128:129])
                    nc.vector.tensor_scalar_mul(
                        self.osb[:, r * 128 : r * 128 + 128],
                        ctxs3[:, r, 0:128],
                        rec[:],
                    )
                    # alternate issue queues so the tail DMAs pipeline
                    deng = (nc.sync, nc.scalar, nc.gpsimd, nc.sync)[r]
                    deng.dma_start(
                        out_d[h, 128 * i : 128 * (i + 1), :],
                        self.osb[:, r * 128 : r * 128 + 128],
                    )
                elif i % 4 == 3:
                    g = i // 4
                    rec = sb.tile([D, 4], F32, tag="rec", bufs=2,
                                  name=f"rec_{h}_{g}")
                    nc.vector.reciprocal(rec[:], ctxs3[:, :, 128])
                    for r in range(4):
                        nc.vector.tensor_scalar_mul(
                            self.osb[:, r * 128 : r * 128 + 128],
                            ctxs3[:, r, 0:128],
                            rec[:, r : r + 1],
                        )
                    nc.sync.dma_start(
                        out_d[h, 512 * g : 512 * (g + 1), :].rearrange(
                            "(b s) e -> s b e", b=4
                        ),
                        self.osb.rearrange("p (b e) -> p b e", b=4),
                    )

            def pv_flush(self):
                """Incrementally emit PV pairs whose expT weights and vsb
                chunks exist; a region's accumulation group stays open in its
                PSUM slot across spans. Region i may open once region i-3 has
                closed (3 rotating slots)."""
                done = self.spans[self.span_i - 1][2] + self.spans[self.span_i - 1][3] \
                    if self.span_i > 0 else 0
                for i in range(NB):
                    if self.closed[i]:
                        continue
                    # only ONE open accumulation group per 2KB PSUM bank:
                    # start=True marks the whole bank's zero-region on trn2,
                    # so a region may open only if its bank has no open group
                    bankid, slot = self._slot(i)
                    if bankid == 1 and self.vrounds < 4:
                        continue
                    cur = self.bank_open.get(bankid)
                    if cur is not None and cur != i:
                        continue
                    j = self.next_pair[i]
                    while (
                        j <= i
                        and OFF[j] + (i - j) * 128 + 128 <= done
                        and j < 4 * self.vrounds
                    ):
                        woff = OFF[j] + (i - j) * 128
                        nc.tensor.matmul(
                            slot,
                            self.expt[:, woff : woff + 128],
                            self.vsb[:, 129 * j : 129 * j + 129],
                            start=(j == 0),
                            stop=(j == i),
                        )
                        j += 1
                    self.next_pair[i] = j
                    if j > i:
                        self._close_pv(i)
                        self.closed[i] = True
                        self.bank_open[bankid] = None
                    elif j > 0:
                        self.bank_open[bankid] = i

            def span_run(self, a, b, flush=True):
                """Emit score matmuls + exp for spans [a, b)."""
                for si in range(a, b):
                    r, ro, st, ln = self.spans[si]
                    reg = regions[r]
                    pos = st
                    while pos < st + ln:
                        # current k-block
                        j = 0
                        while OFF[j + 1] <= pos:
                            j += 1
                        col = ro + (pos - st)
                        nxt = min(
                            OFF[j + 1],
                            st + ln,
                            pos + (512 - (col % 512)),
                        )
                        nc.tensor.matmul(
                            reg[:, col : col + (nxt - pos)],
                            self.kmt[:, 128 * j : 128 * (j + 1)],
                            self.qtb[
                                :,
                                128 * j + (pos - OFF[j]) : 128 * j + (nxt - OFF[j]),
                            ],
                        )
                        pos = nxt
                    self._fire_span(si)
                    self.span_i = si + 1
                    if flush:
                        self.pv_flush()

        e0 = HeadEmitter(0)
        e1 = HeadEmitter(1)

        # ---- software-pipelined drive of the two heads -------------------
        e0.load()
        # junk matmuls fill the PE during the input-DMA window (HAM warmup)
        for _ in range(30):
            nc.tensor.matmul(ctxb[:, 0:128], wup[:], wup[:])
        e0.pro_M()
        e0.kchunk(0)
        e0.load_v()
        e0.span_run(0, 1, flush=False)
        e0.vround(0)
        e0.kchunk(1)
        e0.span_run(1, 2)
        e0.vround(1)
        e0.span_run(2, 3)
        e0.vround(2)
        e0.span_run(3, 4)
        e0.vround(3)
        e0.span_run(4, 5)
        e0.kchunk(2)
        e0.span_run(5, 6)
        e0.kchunk(3)
        # dependency gates: stop the scheduler from hoisting head1's DMA
        # issues into head0's critical input-load window
        nc.gpsimd.tensor_copy(e1.vtb[:, 0:1], e0.expt[:, 0:1])
        nc.gpsimd.tensor_copy(e1.wvb[:, 0:1], e0.expt[:, 0:1])
        e1.load()
        e1.load_v()
        e0.span_run(6, 7)
        e1.pro_M()
        e0.span_run(7, 8)
        e1.kchunk(0)
        e0.span_run(8, 9)
        e1.kchunk(1)
        e0.span_run(9, 10)
        e1.kchunk(2)
        e0.span_run(10, 11)
        e1.kchunk(3)
        e0.span_run(11, 12)
        e1.span_run(0, 1, flush=False)
        e0.pv_flush()  # head0 tail regions
        e1.vround(0)
        e1.span_run(1, 2)
        e1.vround(1)
        e1.span_run(2, 3)
        e1.vround(2)
        e1.span_run(3, 4)
        e1.vround(3)
        e1.span_run(4, len(e1.spans))
        e1.pv_flush()

    nc.compile()
    return nc


_NC_CACHE = None


def _get_program():
    global _NC_CACHE
    if _NC_CACHE is None:
        _NC_CACHE = build_program()
    return _NC_CACHE


def make_in_maps(query_layer, key_layer, value_layer, svd_qk, svd_v):
    bf = ml_dtypes.bfloat16
    qt = np.ascontiguousarray(
        np.asarray(query_layer)[:, 0].transpose(1, 2, 0).astype(bf)
    )
    kt = np.ascontiguousarray(
        np.asarray(key_layer)[:, 0].transpose(1, 2, 0).astype(bf)
    )
    vt = np.ascontiguousarray(
        np.asarray(value_layer)[:, 0].transpose(1, 2, 0).astype(bf)
    )
    wqkt = np.ascontiguousarray(
        np.asarray(svd_qk).transpose(0, 2, 1).astype(bf)
    )
    wv = np.ascontiguousarray(np.asarray(svd_v).astype(bf))

    in_maps = []
    for c in range(NCORES):
        hs = slice(c * HPC, (c + 1) * HPC)
        in_maps.append(
            {
                "qt": qt[hs],
                "kt": kt[hs],
                "vt": vt[hs],
                "wqkt": wqkt[hs],
                "wv": wv[hs],
            }
        )
    return in_maps


def assemble_output(results):
    out = np.empty((S, B, H * D), dtype=np.float32)
    for c in range(NCORES):
        o = results[c]["out"]  # [HPC, S, D]
        for hl in range(HPC):
            h = c * HPC + hl
            out[:, 0, h * D : (h + 1) * D] = o[hl]
    return out


def kernel(query_layer, key_layer, value_layer, attention_mask, svd_qk, svd_v):
    nc = _get_program()
    in_maps = make_in_maps(query_layer, key_layer, value_layer, svd_qk, svd_v)
    res = run_bass_kernel_spmd(nc, in_maps, list(range(NCORES))).results
    return assemble_output(res)
